# revision 2
# baseline (speedup 1.0000x reference)
"""AdEx neuron simulation kernel for 8 Trainium2 NeuronCores.

Reference semantics (per timestep, fp32):
    exp_term = Delta_T * exp((V - V_T)/Delta_T)
    V <- V + dt/tau_m * (-(V-E_L) + exp_term - R*w + R*I)
    spk = V >= V_spike ; V <- V_reset where spk
(w stays identically 0 for the a=0, b=0 parameterization.)

Kernel formulation (state Y = V - V_reset, c = dt/tau_m, A = 1-c):
    e_t = exp(s*Y + b)            s = 1/Delta_T, b = (V_reset-V_T)/Delta_T + ln(c*Delta_T)
    u_t = A*Y_{t-1} + J_t + e_t   J_t = c*R*I_t + c*(E_L - V_reset)   (host-precomputed)
    Y_t = u_t if u_t < thr else 0 thr = V_spike - V_reset
    spike_t = (Y_t == 0)          extracted in bulk per chunk

Sharding: batch rows 4k..4k+3 -> core k (4096 neurons/core, [128 x 32] tiles),
serial 2000-step loop per core; no cross-core communication.
Engines: ScalarE does exp; PE accumulates J_t + e_t into PSUM via identity
matmuls; VectorE does the affine+select (2 scalar_tensor_tensor ops).
"""

import numpy as np

B, T, D = 32, 2000, 1024
N_CORES = 8
BPC = B // N_CORES            # batch rows per core
NPC = BPC * D                 # neurons per core = 4096
W = NPC // 128                # free-dim width = 32


def _build_graph(consts, G=1, CH=125, steps=T):
    import concourse.bass as bass
    import concourse.mybir as mybir

    A, s, bias, thr = consts["A"], consts["s"], consts["bias"], consts["thr"]
    y0 = consts["y0"]
    f32 = mybir.dt.float32
    NCH = steps // CH
    assert steps % CH == 0
    GW = W // G
    assert W % G == 0

    nc = bass.Bass()

    # init constants in SBUF
    bias_t = nc.alloc_sbuf_tensor("expbias", [128, 1], f32)
    nc.gpsimd.memset(bias_t.ap(), float(bias))
    yinit = nc.alloc_sbuf_tensor("yinit", [128, W], f32)
    nc.gpsimd.memset(yinit.ap(), float(y0))
    nc.all_engine_barrier()

    J_ext = nc.declare_dram_parameter("J", [128, steps, W], f32, isOutput=False)
    eye_ext = nc.declare_dram_parameter("eye", [128, 128], f32, isOutput=False)
    spk_ext = nc.declare_dram_parameter("spk", [128, steps, W], f32, isOutput=True)

    with (
        nc.sbuf_tensor([128, 2, CH, W], f32) as jbuf,
        nc.sbuf_tensor([128, 128], f32) as eye,
        nc.sbuf_tensor([128, 2, CH, W], f32) as hist,
        nc.sbuf_tensor([128, 2, CH, W], f32) as spkst,
        nc.sbuf_tensor([128, 2, W], f32) as ebuf,
        nc.sbuf_tensor([128, 2, W], f32) as ubuf,
        nc.psum_tensor([128, 2, W], f32) as psum,
        nc.semaphore() as dmaJ_sem,
        nc.semaphore() as dmaS_sem,
        nc.semaphore() as dmaE_sem,
        nc.semaphore() as spk_sem,
        nc.Block() as block,
    ):
        act_sems = [nc.semaphore(f"act_sem{g}").__enter__() for g in range(G)]
        pe_sems = [nc.semaphore(f"pe_sem{g}").__enter__() for g in range(G)]
        dve_sems = [nc.semaphore(f"dve_sem{g}").__enter__() for g in range(G)]

        def gsl(g):
            return slice(g * GW, (g + 1) * GW)

        def yprev(t, g):
            if t == 0:
                return yinit.ap()[:, gsl(g)]
            tm = t - 1
            return hist[:, (tm // CH) % 2, tm % CH, gsl(g)]

        @block.sync
        def _(sync):
            sync.dma_start(eye[:], eye_ext[:]).then_inc(dmaE_sem, 16)
            # prefetch the first two J chunks
            for ci in range(min(2, NCH)):
                sync.dma_start(
                    jbuf[:, ci % 2], J_ext[:, ci * CH:(ci + 1) * CH]
                ).then_inc(dmaJ_sem, 16)
            for ci in range(NCH):
                # write back spike chunk ci once extracted
                sync.wait_ge(spk_sem, ci + 1)
                sync.dma_start(
                    spk_ext[:, ci * CH:(ci + 1) * CH], spkst[:, ci % 2]
                ).then_inc(dmaS_sem, 16)
                # prefetch J chunk ci+2 (reuses buffer of chunk ci, now consumed)
                if ci + 2 < NCH:
                    for g in range(G):
                        sync.wait_ge(pe_sems[g], (ci + 1) * CH)
                    sync.dma_start(
                        jbuf[:, ci % 2], J_ext[:, (ci + 2) * CH:(ci + 3) * CH]
                    ).then_inc(dmaJ_sem, 16)

        @block.tensor
        def _(tensor):
            tensor.wait_ge(dmaE_sem, 16)
            for t in range(steps):
                ci = t // CH
                if t % CH == 0:
                    tensor.wait_ge(dmaJ_sem, 16 * (ci + 1))
                for g in range(G):
                    if t >= 2:
                        # psum[t%2] WAR: DVE1 of step t-2 has read it
                        tensor.wait_ge(dve_sems[g], t - 1)
                    nc.tensor.matmul(
                        psum[:, t % 2, gsl(g)], eye[:],
                        jbuf[:, ci % 2, t % CH, gsl(g)],
                        start=True, stop=False,
                    )
                    tensor.wait_ge(act_sems[g], t + 1)
                    nc.tensor.matmul(
                        psum[:, t % 2, gsl(g)], eye[:],
                        ebuf[:, t % 2, gsl(g)],
                        start=False, stop=True,
                    ).then_inc(pe_sems[g], 1)

        @block.scalar
        def _(scalar):
            for t in range(steps):
                for g in range(G):
                    if t >= 1:
                        scalar.wait_ge(dve_sems[g], t)
                    if t >= 2:
                        # ebuf[t%2] WAR: PE mm2 of step t-2 has read it
                        scalar.wait_ge(pe_sems[g], t - 1)
                    nc.scalar.activation(
                        ebuf[:, t % 2, gsl(g)], yprev(t, g),
                        mybir.ActivationFunctionType.Exp,
                        bias=bias_t.ap(), scale=float(s),
                    ).then_inc(act_sems[g], 1)

        @block.vector
        def _(vector):
            for t in range(steps):
                ci = t // CH
                for g in range(G):
                    vector.wait_ge(pe_sems[g], t + 1)
                    nc.vector.scalar_tensor_tensor(
                        ubuf[:, t % 2, gsl(g)], yprev(t, g), float(A),
                        psum[:, t % 2, gsl(g)],
                        op0=mybir.AluOpType.mult, op1=mybir.AluOpType.add,
                    )
                    nc.vector.scalar_tensor_tensor(
                        hist[:, ci % 2, t % CH, gsl(g)],
                        ubuf[:, t % 2, gsl(g)], float(thr),
                        ubuf[:, t % 2, gsl(g)],
                        op0=mybir.AluOpType.is_lt, op1=mybir.AluOpType.mult,
                    ).then_inc(dve_sems[g], 1)
                if t % CH == CH - 1:
                    # bulk spike extraction for chunk ci
                    if ci >= 2:
                        vector.wait_ge(dmaS_sem, 16 * (ci - 1))
                    nc.vector.tensor_scalar(
                        spkst[:, ci % 2], hist[:, ci % 2], 0.0, None,
                        mybir.AluOpType.is_equal,
                    ).then_inc(spk_sem, 1)

        for sem in act_sems + pe_sems + dve_sems:
            pass  # context-managed via __enter__; freed at process exit

    return nc


def _derive_consts(params):
    tau_m, E_L, V_T, Delta_T, R, tau_w, a, b, V_reset, V_spike, dt = [
        float(x) for x in params
    ]
    c = dt / tau_m
    return dict(
        A=np.float32(1.0 - c),
        s=np.float32(1.0 / Delta_T),
        bias=np.float32(np.log(c * Delta_T) + (V_reset - V_T) / Delta_T),
        thr=np.float32(V_spike - V_reset),
        y0=np.float32(E_L - V_reset),
        cR=np.float32(c * R),
        Jc=np.float32(c * (E_L - V_reset)),
        a=a, b=b,
    )


def _numpy_fallback(I_seq, params):
    # general-parameter reference port (slow, CPU); used only if a != 0 or b != 0
    tau_m, E_L, V_T, Delta_T, R, tau_w, a, b, V_reset, V_spike, dt = [
        np.float32(x) for x in params
    ]
    Bs, Ts, Ds = I_seq.shape
    I = I_seq.transpose(1, 0, 2).reshape(Ts, Bs * Ds)
    V = np.full(Bs * Ds, E_L, dtype=np.float32)
    w = np.zeros(Bs * Ds, dtype=np.float32)
    out = np.zeros((Ts, Bs * Ds), dtype=np.float32)
    for t in range(Ts):
        exp_term = Delta_T * np.exp((V - V_T) / Delta_T)
        dV = (-(V - E_L) + exp_term - R * w + R * I[t]) / tau_m
        V = V + dt * dV
        dw = (a * (V - E_L) - w) / tau_w
        w = w + dt * dw
        spk = (V >= V_spike).astype(np.float32)
        V = np.where(spk > 0, V_reset, V)
        w = np.where(spk > 0, w + b, w)
        out[t] = spk
    return out.reshape(Ts, Bs, Ds).transpose(1, 0, 2)


_CACHE = {}


def kernel(I_seq, params):
    I_seq = np.asarray(I_seq, dtype=np.float32)
    params = np.asarray(params, dtype=np.float32)
    consts = _derive_consts(params)
    if consts["a"] != 0.0 or consts["b"] != 0.0:
        return _numpy_fallback(I_seq, params)

    from concourse.bass_utils import run_bass_kernel_spmd

    # host-side input prep: J = cR*I + Jc, laid out [128, T, 32] per core
    J = (consts["cR"] * I_seq + consts["Jc"]).astype(np.float32)
    eye = np.eye(128, dtype=np.float32)
    in_maps = []
    for k in range(N_CORES):
        jk = J[BPC * k: BPC * (k + 1)]                       # [4, T, 1024]
        jk = jk.reshape(BPC, T, W, D // W // 1)              # [4, T, 32, 32]
        jk = np.ascontiguousarray(jk.transpose(0, 2, 1, 3))  # [4, 32, T, 32]
        jk = jk.reshape(128, T, W)
        in_maps.append({"J": jk, "eye": eye})

    key = tuple(np.asarray(params).tobytes())
    if key not in _CACHE:
        _CACHE[key] = _build_graph(consts, G=1, CH=125)
    nc = _CACHE[key]

    res = run_bass_kernel_spmd(nc, in_maps, core_ids=list(range(N_CORES)))

    out = np.empty((B, T, D), dtype=np.float32)
    for k in range(N_CORES):
        sk = res.results[k]["spk"]                           # [128, T, 32]
        sk = sk.reshape(BPC, W, T, D // W)                   # [4, 32, T, 32]
        sk = sk.transpose(0, 2, 1, 3).reshape(BPC, T, D)     # [4, T, 1024]
        out[BPC * k: BPC * (k + 1)] = sk
    return out


# revision 3
# speedup vs baseline: 1.9754x; 1.9754x over previous
"""AdEx neuron simulation kernel for 8 Trainium2 NeuronCores.

Reference semantics (per timestep, fp32):
    exp_term = Delta_T * exp((V - V_T)/Delta_T)
    V <- V + dt/tau_m * (-(V-E_L) + exp_term - R*w + R*I)
    spk = V >= V_spike ; V <- V_reset where spk
(w stays identically 0 for the a=0, b=0 parameterization.)

Kernel formulation (state Y = V - V_reset, c = dt/tau_m, A = 1-c):
    e_t = exp(s*Y + b)            s = 1/Delta_T, b = (V_reset-V_T)/Delta_T + ln(c*Delta_T)
    u_t = A*Y_{t-1} + J_t + e_t   J_t = c*R*I_t + c*(E_L - V_reset)   (host-precomputed)
    Y_t = u_t if u_t < thr else 0 thr = V_spike - V_reset
    spike_t = (Y_t == 0)          extracted in bulk per chunk

Sharding: batch rows 4k..4k+3 -> core k (4096 neurons/core, [128 x 32] tiles),
serial 2000-step loop per core; no cross-core communication.
Engines: ScalarE does exp; PE accumulates J_t + e_t into PSUM via identity
matmuls; VectorE does the affine+select (2 scalar_tensor_tensor ops).
"""

import numpy as np

B, T, D = 32, 2000, 1024
N_CORES = 8
BPC = B // N_CORES            # batch rows per core
NPC = BPC * D                 # neurons per core = 4096
W = NPC // 128                # free-dim width = 32


def _build_graph(consts, G=1, CH=125, steps=T):
    import concourse.bass as bass
    import concourse.mybir as mybir

    A, s, bias, thr = consts["A"], consts["s"], consts["bias"], consts["thr"]
    y0 = consts["y0"]
    f32 = mybir.dt.float32
    NCH = steps // CH
    assert steps % CH == 0
    GW = W // G
    assert W % G == 0

    nc = bass.Bass()

    # init constants in SBUF
    bias_t = nc.alloc_sbuf_tensor("expbias", [128, 1], f32)
    nc.gpsimd.memset(bias_t.ap(), float(bias))
    yinit = nc.alloc_sbuf_tensor("yinit", [128, W], f32)
    nc.gpsimd.memset(yinit.ap(), float(y0))
    nc.all_engine_barrier()

    J_ext = nc.declare_dram_parameter("J", [128, steps, W], f32, isOutput=False)
    spk_ext = nc.declare_dram_parameter("spk", [128, steps, W], f32, isOutput=True)

    with (
        nc.sbuf_tensor([128, 2, CH, W], f32) as jbuf,
        nc.sbuf_tensor([128, 2, CH, W], f32) as hist,
        nc.sbuf_tensor([128, 2, CH, W], f32) as spkst,
        nc.sbuf_tensor([128, 2, W], f32) as ebuf,
        nc.sbuf_tensor([128, 2, W], f32) as hbuf,
        nc.sbuf_tensor([128, 2, W], f32) as ubuf,
        nc.semaphore("dmaJ_sem") as dmaJ_sem,
        nc.semaphore("dmaS_sem") as dmaS_sem,
        nc.semaphore("spk_sem") as spk_sem,
        nc.semaphore("act_sem") as act_sem,
        nc.semaphore("dve_sem") as dve_sem,
        nc.Block() as block,
    ):
        def yprev(t):
            if t == 0:
                return yinit.ap()
            tm = t - 1
            return hist[:, (tm // CH) % 2, tm % CH]

        @block.sync
        def _(sync):
            # prefetch the first two J chunks
            for ci in range(min(2, NCH)):
                sync.dma_start(
                    jbuf[:, ci % 2], J_ext[:, ci * CH:(ci + 1) * CH]
                ).then_inc(dmaJ_sem, 16)
            for ci in range(NCH):
                # write back spike chunk ci once extracted
                sync.wait_ge(spk_sem, ci + 1)
                sync.dma_start(
                    spk_ext[:, ci * CH:(ci + 1) * CH], spkst[:, ci % 2]
                ).then_inc(dmaS_sem, 16)
                # prefetch J chunk ci+2 (reuses buffer of chunk ci, consumed
                # once DVE finished chunk ci: dve_sem >= (ci+1)*CH)
                if ci + 2 < NCH:
                    sync.dma_start(
                        jbuf[:, ci % 2], J_ext[:, (ci + 2) * CH:(ci + 3) * CH]
                    ).then_inc(dmaJ_sem, 16)

        @block.scalar
        def _(scalar):
            for t in range(steps):
                if t >= 1:
                    scalar.wait_ge(dve_sem, t)
                nc.scalar.activation(
                    ebuf[:, t % 2], yprev(t),
                    mybir.ActivationFunctionType.Exp,
                    bias=bias_t.ap(), scale=float(s),
                ).then_inc(act_sem, 1)

        @block.vector
        def _(vector):
            # h_0 prologue: h[0] = A*yinit + J_0
            vector.wait_ge(dmaJ_sem, 16)
            nc.vector.scalar_tensor_tensor(
                hbuf[:, 0], yinit.ap(), float(A), jbuf[:, 0, 0],
                op0=mybir.AluOpType.mult, op1=mybir.AluOpType.add,
            )
            for t in range(steps):
                ci = t // CH
                # on-chain: u_t = h_t + e_t ; Y_t = (u_t < thr) * u_t
                vector.wait_ge(act_sem, t + 1)
                nc.vector.tensor_tensor(
                    ubuf[:, t % 2], hbuf[:, t % 2], ebuf[:, t % 2],
                    op=mybir.AluOpType.add,
                )
                nc.vector.scalar_tensor_tensor(
                    hist[:, ci % 2, t % CH],
                    ubuf[:, t % 2], float(thr), ubuf[:, t % 2],
                    op0=mybir.AluOpType.is_lt, op1=mybir.AluOpType.mult,
                ).then_inc(dve_sem, 1)
                # off-chain (overlaps ACT of step t+1): h_{t+1} = A*Y_t + J_{t+1}
                if t + 1 < steps:
                    tn = t + 1
                    cn = tn // CH
                    if tn % CH == 0:
                        vector.wait_ge(dmaJ_sem, 16 * (cn + 1))
                    nc.vector.scalar_tensor_tensor(
                        hbuf[:, tn % 2], hist[:, ci % 2, t % CH], float(A),
                        jbuf[:, cn % 2, tn % CH],
                        op0=mybir.AluOpType.mult, op1=mybir.AluOpType.add,
                    )
                if t % CH == CH - 1:
                    # bulk spike extraction for chunk ci
                    if ci >= 2:
                        vector.wait_ge(dmaS_sem, 16 * (ci - 1))
                    nc.vector.tensor_scalar(
                        spkst[:, ci % 2], hist[:, ci % 2], 0.0, None,
                        mybir.AluOpType.is_equal,
                    ).then_inc(spk_sem, 1)

    return nc


def _derive_consts(params):
    tau_m, E_L, V_T, Delta_T, R, tau_w, a, b, V_reset, V_spike, dt = [
        float(x) for x in params
    ]
    c = dt / tau_m
    return dict(
        A=np.float32(1.0 - c),
        s=np.float32(1.0 / Delta_T),
        bias=np.float32(np.log(c * Delta_T) + (V_reset - V_T) / Delta_T),
        thr=np.float32(V_spike - V_reset),
        y0=np.float32(E_L - V_reset),
        cR=np.float32(c * R),
        Jc=np.float32(c * (E_L - V_reset)),
        a=a, b=b,
    )


def _numpy_fallback(I_seq, params):
    # general-parameter reference port (slow, CPU); used only if a != 0 or b != 0
    tau_m, E_L, V_T, Delta_T, R, tau_w, a, b, V_reset, V_spike, dt = [
        np.float32(x) for x in params
    ]
    Bs, Ts, Ds = I_seq.shape
    I = I_seq.transpose(1, 0, 2).reshape(Ts, Bs * Ds)
    V = np.full(Bs * Ds, E_L, dtype=np.float32)
    w = np.zeros(Bs * Ds, dtype=np.float32)
    out = np.zeros((Ts, Bs * Ds), dtype=np.float32)
    for t in range(Ts):
        exp_term = Delta_T * np.exp((V - V_T) / Delta_T)
        dV = (-(V - E_L) + exp_term - R * w + R * I[t]) / tau_m
        V = V + dt * dV
        dw = (a * (V - E_L) - w) / tau_w
        w = w + dt * dw
        spk = (V >= V_spike).astype(np.float32)
        V = np.where(spk > 0, V_reset, V)
        w = np.where(spk > 0, w + b, w)
        out[t] = spk
    return out.reshape(Ts, Bs, Ds).transpose(1, 0, 2)


_CACHE = {}


def kernel(I_seq, params):
    I_seq = np.asarray(I_seq, dtype=np.float32)
    params = np.asarray(params, dtype=np.float32)
    consts = _derive_consts(params)
    if consts["a"] != 0.0 or consts["b"] != 0.0:
        return _numpy_fallback(I_seq, params)

    from concourse.bass_utils import run_bass_kernel_spmd

    # host-side input prep: J = cR*I + Jc, laid out [128, T, 32] per core
    J = (consts["cR"] * I_seq + consts["Jc"]).astype(np.float32)
    eye = np.eye(128, dtype=np.float32)
    in_maps = []
    for k in range(N_CORES):
        jk = J[BPC * k: BPC * (k + 1)]                       # [4, T, 1024]
        jk = jk.reshape(BPC, T, W, D // W // 1)              # [4, T, 32, 32]
        jk = np.ascontiguousarray(jk.transpose(0, 2, 1, 3))  # [4, 32, T, 32]
        jk = jk.reshape(128, T, W)
        in_maps.append({"J": jk, "eye": eye})

    key = tuple(np.asarray(params).tobytes())
    if key not in _CACHE:
        _CACHE[key] = _build_graph(consts, G=1, CH=125)
    nc = _CACHE[key]

    res = run_bass_kernel_spmd(nc, in_maps, core_ids=list(range(N_CORES)))

    out = np.empty((B, T, D), dtype=np.float32)
    for k in range(N_CORES):
        sk = res.results[k]["spk"]                           # [128, T, 32]
        sk = sk.reshape(BPC, W, T, D // W)                   # [4, 32, T, 32]
        sk = sk.transpose(0, 2, 1, 3).reshape(BPC, T, D)     # [4, T, 1024]
        out[BPC * k: BPC * (k + 1)] = sk
    return out


# revision 4
# speedup vs baseline: 2.3934x; 1.2117x over previous
"""AdEx neuron simulation kernel for 8 Trainium2 NeuronCores.

Reference semantics (per timestep, fp32):
    exp_term = Delta_T * exp((V - V_T)/Delta_T)
    V <- V + dt/tau_m * (-(V-E_L) + exp_term - R*w + R*I)
    spk = V >= V_spike ; V <- V_reset where spk
(w stays identically 0 for the a=0, b=0 parameterization.)

Kernel formulation (state Y = V - V_reset, c = dt/tau_m, A = 1-c):
    e_t = exp(s*Y + b)            s = 1/Delta_T, b = (V_reset-V_T)/Delta_T + ln(c*Delta_T)
    u_t = A*Y_{t-1} + J_t + e_t   J_t = c*R*I_t + c*(E_L - V_reset)   (host-precomputed)
    Y_t = u_t if u_t < thr else 0 thr = V_spike - V_reset
    spike_t = (Y_t == 0)          extracted in bulk per chunk

Sharding: batch rows 4k..4k+3 -> core k (4096 neurons/core, [128 x 32] tiles),
serial 2000-step loop per core; no cross-core communication.
Engines: ScalarE does exp; PE accumulates J_t + e_t into PSUM via identity
matmuls; VectorE does the affine+select (2 scalar_tensor_tensor ops).
"""

import numpy as np

B, T, D = 32, 2000, 1024
N_CORES = 8
BPC = B // N_CORES            # batch rows per core
NPC = BPC * D                 # neurons per core = 4096
W = NPC // 128                # free-dim width = 32


def _build_graph(consts, G=1, CH=125, steps=T):
    import concourse.bass as bass
    import concourse.mybir as mybir

    A, s, bias, thr = consts["A"], consts["s"], consts["bias"], consts["thr"]
    y0 = consts["y0"]
    f32 = mybir.dt.float32
    NCH = steps // CH
    assert steps % CH == 0
    GW = W // G
    assert W % G == 0

    nc = bass.Bass()

    # init constants in SBUF
    bias_t = nc.alloc_sbuf_tensor("expbias", [128, 1], f32)
    nc.gpsimd.memset(bias_t.ap(), float(bias))
    yinit = nc.alloc_sbuf_tensor("yinit", [128, W], f32)
    nc.gpsimd.memset(yinit.ap(), float(y0))
    nc.all_engine_barrier()

    J_ext = nc.declare_dram_parameter("J", [128, steps, W], f32, isOutput=False)
    spk_ext = nc.declare_dram_parameter("spk", [128, steps, W], f32, isOutput=True)

    with (
        nc.sbuf_tensor([128, 2, CH, W], f32) as jbuf,
        nc.sbuf_tensor([128, 2, CH, W], f32) as hist,
        nc.sbuf_tensor([128, 2, CH, W], f32) as spkst,
        nc.sbuf_tensor([128, 2, W], f32) as ebuf,
        nc.sbuf_tensor([128, 2, W], f32) as hbuf,
        nc.sbuf_tensor([128, 2, W], f32) as ubuf,
        nc.semaphore("dmaJ_sem") as dmaJ_sem,
        nc.semaphore("dmaS_sem") as dmaS_sem,
        nc.semaphore("spk_sem") as spk_sem,
        nc.semaphore("act_sem") as act_sem,
        nc.semaphore("dve_sem") as dve_sem,
        nc.Block() as block,
    ):
        def yprev(t):
            if t == 0:
                return yinit.ap()
            tm = t - 1
            return hist[:, (tm // CH) % 2, tm % CH]

        @block.sync
        def _(sync):
            # prefetch the first two J chunks
            for ci in range(min(2, NCH)):
                sync.dma_start(
                    jbuf[:, ci % 2], J_ext[:, ci * CH:(ci + 1) * CH]
                ).then_inc(dmaJ_sem, 16)
            for ci in range(NCH):
                # write back spike chunk ci once extracted
                sync.dma_start(
                    spk_ext[:, ci * CH:(ci + 1) * CH], spkst[:, ci % 2]
                )._wait_ge(spk_sem, ci + 1).then_inc(dmaS_sem, 16)
                # prefetch J chunk ci+2 (reuses buffer of chunk ci, consumed
                # by the time DVE's chunk-ci spikes are extracted)
                if ci + 2 < NCH:
                    sync.dma_start(
                        jbuf[:, ci % 2], J_ext[:, (ci + 2) * CH:(ci + 3) * CH]
                    ).then_inc(dmaJ_sem, 16)

        @block.scalar
        def _(scalar):
            for t in range(steps):
                ins = nc.scalar.activation(
                    ebuf[:, t % 2], yprev(t),
                    mybir.ActivationFunctionType.Exp,
                    bias=bias_t.ap(), scale=float(s),
                ).then_inc(act_sem, 1)
                if t >= 1:
                    ins._wait_ge(dve_sem, t)

        @block.vector
        def _(vector):
            # h_0 prologue: h[0] = A*yinit + J_0
            nc.vector.scalar_tensor_tensor(
                hbuf[:, 0], yinit.ap(), float(A), jbuf[:, 0, 0],
                op0=mybir.AluOpType.mult, op1=mybir.AluOpType.add,
            )._wait_ge(dmaJ_sem, 16)
            for t in range(steps):
                ci = t // CH
                # on-chain: u_t = h_t + e_t ; Y_t = (u_t < thr) * u_t
                nc.vector.scalar_tensor_tensor(
                    ubuf[:, t % 2], hbuf[:, t % 2], 0.0, ebuf[:, t % 2],
                    op0=mybir.AluOpType.add, op1=mybir.AluOpType.add,
                )._wait_ge(act_sem, t + 1)
                nc.vector.scalar_tensor_tensor(
                    hist[:, ci % 2, t % CH],
                    ubuf[:, t % 2], float(thr), ubuf[:, t % 2],
                    op0=mybir.AluOpType.is_lt, op1=mybir.AluOpType.mult,
                ).then_inc(dve_sem, 1)
                # off-chain (overlaps ACT of step t+1): h_{t+1} = A*Y_t + J_{t+1}
                if t + 1 < steps:
                    tn = t + 1
                    cn = tn // CH
                    ins = nc.vector.scalar_tensor_tensor(
                        hbuf[:, tn % 2], hist[:, ci % 2, t % CH], float(A),
                        jbuf[:, cn % 2, tn % CH],
                        op0=mybir.AluOpType.mult, op1=mybir.AluOpType.add,
                    )
                    if tn % CH == 0:
                        ins._wait_ge(dmaJ_sem, 16 * (cn + 1))
                if t % CH == CH - 1:
                    # bulk spike extraction for chunk ci
                    ins = nc.vector.tensor_scalar(
                        spkst[:, ci % 2], hist[:, ci % 2], 0.0, None,
                        mybir.AluOpType.is_equal,
                    ).then_inc(spk_sem, 1)
                    if ci >= 2:
                        ins._wait_ge(dmaS_sem, 16 * (ci - 1))

    return nc


def _derive_consts(params):
    tau_m, E_L, V_T, Delta_T, R, tau_w, a, b, V_reset, V_spike, dt = [
        float(x) for x in params
    ]
    c = dt / tau_m
    return dict(
        A=np.float32(1.0 - c),
        s=np.float32(1.0 / Delta_T),
        bias=np.float32(np.log(c * Delta_T) + (V_reset - V_T) / Delta_T),
        thr=np.float32(V_spike - V_reset),
        y0=np.float32(E_L - V_reset),
        cR=np.float32(c * R),
        Jc=np.float32(c * (E_L - V_reset)),
        a=a, b=b,
    )


def _numpy_fallback(I_seq, params):
    # general-parameter reference port (slow, CPU); used only if a != 0 or b != 0
    tau_m, E_L, V_T, Delta_T, R, tau_w, a, b, V_reset, V_spike, dt = [
        np.float32(x) for x in params
    ]
    Bs, Ts, Ds = I_seq.shape
    I = I_seq.transpose(1, 0, 2).reshape(Ts, Bs * Ds)
    V = np.full(Bs * Ds, E_L, dtype=np.float32)
    w = np.zeros(Bs * Ds, dtype=np.float32)
    out = np.zeros((Ts, Bs * Ds), dtype=np.float32)
    for t in range(Ts):
        exp_term = Delta_T * np.exp((V - V_T) / Delta_T)
        dV = (-(V - E_L) + exp_term - R * w + R * I[t]) / tau_m
        V = V + dt * dV
        dw = (a * (V - E_L) - w) / tau_w
        w = w + dt * dw
        spk = (V >= V_spike).astype(np.float32)
        V = np.where(spk > 0, V_reset, V)
        w = np.where(spk > 0, w + b, w)
        out[t] = spk
    return out.reshape(Ts, Bs, Ds).transpose(1, 0, 2)


_CACHE = {}


def kernel(I_seq, params):
    I_seq = np.asarray(I_seq, dtype=np.float32)
    params = np.asarray(params, dtype=np.float32)
    consts = _derive_consts(params)
    if consts["a"] != 0.0 or consts["b"] != 0.0:
        return _numpy_fallback(I_seq, params)

    from concourse.bass_utils import run_bass_kernel_spmd

    # host-side input prep: J = cR*I + Jc, laid out [128, T, 32] per core
    J = (consts["cR"] * I_seq + consts["Jc"]).astype(np.float32)
    eye = np.eye(128, dtype=np.float32)
    in_maps = []
    for k in range(N_CORES):
        jk = J[BPC * k: BPC * (k + 1)]                       # [4, T, 1024]
        jk = jk.reshape(BPC, T, W, D // W // 1)              # [4, T, 32, 32]
        jk = np.ascontiguousarray(jk.transpose(0, 2, 1, 3))  # [4, 32, T, 32]
        jk = jk.reshape(128, T, W)
        in_maps.append({"J": jk, "eye": eye})

    key = tuple(np.asarray(params).tobytes())
    if key not in _CACHE:
        _CACHE[key] = _build_graph(consts, G=1, CH=125)
    nc = _CACHE[key]

    res = run_bass_kernel_spmd(nc, in_maps, core_ids=list(range(N_CORES)))

    out = np.empty((B, T, D), dtype=np.float32)
    for k in range(N_CORES):
        sk = res.results[k]["spk"]                           # [128, T, 32]
        sk = sk.reshape(BPC, W, T, D // W)                   # [4, 32, T, 32]
        sk = sk.transpose(0, 2, 1, 3).reshape(BPC, T, D)     # [4, T, 1024]
        out[BPC * k: BPC * (k + 1)] = sk
    return out
